# revision 39
# baseline (speedup 1.0000x reference)
"""Butterfly (Givens) rotation network on TRN2, 8 NeuronCores.

Algorithm
---------
x: (8192, 4096) f32. 12 butterfly layers; layer l rotates pairs of features
differing in bit l of the feature index. Split into two linear stages:

  Stage A = layers 0-6: features mix only within 128-wide blocks b (bits 0-6)
            -> per-block 128x128 matrix A_b.
  Stage B = layers 7-11: features mix only across blocks at fixed within-block
            position p (bits 7-11) -> per-p 32x32 matrix B_p; grouping 4
            consecutive p per 128-partition tile gives block-diag 128x128.

Per 128-row tile (rows on partitions), all on the TensorEngine:
  pass1: per block b: PE-transpose x_b -> xT_b [f',r] (PSUM->SBUF copy),
         MM out[r,fo] = sum_f' xT_b[f',r] * A_bT[f',fo]  (lhsT=xT_b, rhs=A_bT)
         scatter-copy PSUM->SBUF into Y with f~ = (p//4)*128 + (p%4)*32 + b.
  pass2: per f~-tile t: PE-transpose Y_t -> z [f~',r],
         MM out[r,n] = sum z[f~',r] * BDT_t[f~',n], scatter-copy to natural
         feature order, DMA out.

Sharding: data-parallel over rows, 1024 rows/core; matrices replicated.
"""

import math
import numpy as np

DIM = 4096
NL = 12
NB = 32          # 128-wide feature blocks
ROWS = 8192
NCORES = 8
RPC = ROWS // NCORES     # rows per core
NT = RPC // 128          # 128-row tiles per core


# ---------------------------------------------------------------- host math

def _butterfly_np(x, angles):
    """float64 numpy copy of the reference butterfly."""
    x = np.asarray(x, np.float64)
    angles = np.asarray(angles, np.float64)
    B, d = x.shape
    for l in range(angles.shape[0]):
        stride = 2 ** l
        nblocks = d // (2 * stride)
        xr = x.reshape(B, nblocks, 2, stride)
        c = np.cos(angles[l]).reshape(nblocks, stride)
        s = np.sin(angles[l]).reshape(nblocks, stride)
        xi = xr[:, :, 0, :].copy()
        xj = xr[:, :, 1, :].copy()
        x = np.stack([c * xi + s * xj, -s * xi + c * xj], axis=2).reshape(B, d)
    return x


def _build_mats(angles):
    """Returns (amats, bmats) each [128, 4096] f32 in SBUF-ready layout."""
    angles = np.asarray(angles, np.float64)
    amats = np.zeros((128, DIM), np.float64)
    for b in range(NB):
        # A_bT[f_in, f_out]: butterfly of identity rows = F^T for this block
        amats[:, 128 * b:128 * b + 128] = _butterfly_np(
            np.eye(128), angles[0:7, 64 * b:64 * b + 64])
    bmats = np.zeros((128, DIM), np.float64)
    for t in range(32):
        for pl in range(4):
            p = 4 * t + pl
            BpT = _butterfly_np(np.eye(32), angles[7:12, p::128])
            bmats[32 * pl:32 * pl + 32, 128 * t + 32 * pl:128 * t + 32 * pl + 32] = BpT
    return amats.astype(np.float32), bmats.astype(np.float32)


# ---------------------------------------------------------------- bass kernel

def _emit_kernel(ctx, tc, out, x, amats, bmats, ident):
    import concourse.bass as bass
    import concourse.mybir as mybir

    nc = tc.nc
    f32 = mybir.dt.float32

    consts = ctx.enter_context(tc.tile_pool(name="consts", bufs=1))
    xin = ctx.enter_context(tc.tile_pool(name="xin", bufs=3))
    ystage = ctx.enter_context(tc.tile_pool(name="ystage", bufs=3))
    ostage = ctx.enter_context(tc.tile_pool(name="ostage", bufs=3))
    sbst = ctx.enter_context(tc.tile_pool(name="sbst", bufs=6))
    psA = ctx.enter_context(tc.tile_pool(name="psA", bufs=4, space="PSUM"))
    psB = ctx.enter_context(tc.tile_pool(name="psB", bufs=4, space="PSUM"))

    am = consts.tile([128, DIM], f32, tag="amats")
    bm = consts.tile([128, DIM], f32, tag="bmats")
    idt = consts.tile([128, 128], f32, tag="ident")
    nc.sync.dma_start(idt[:], ident[:])

    # Greedy least-loaded assignment of PSUM->SBUF copies to DVE/ACT,
    # using measured per-copy costs (ns) for [128,512] fp32 from PSUM.
    load = {"dve": 0.0, "act": 0.0}
    cost = {("dve", "plain"): 685, ("dve", "scatter"): 700,
            ("act", "plain"): 570, ("act", "scatter"): 1127}

    def copy(dst, src, kind="plain"):
        eng = min(("dve", "act"), key=lambda e: load[e] + cost[(e, kind)])
        load[eng] += cost[(eng, kind)]
        (nc.vector.tensor_copy if eng == "dve" else nc.scalar.copy)(dst, src)

    for i in range(NT):
        xt = xin.tile([128, DIM], f32, tag="xt")
        if i == 0:
            # first tile: fine-grained x/amats chunk interleave so the very
            # first transposes and stage-A matmuls start as early as possible
            for c in range(8):
                nc.sync.dma_start(xt[:, 512 * c:512 * (c + 1)],
                                  x[0:128, 512 * c:512 * (c + 1)])
                nc.sync.dma_start(am[:, 512 * c:512 * (c + 1)],
                                  amats[:, 512 * c:512 * (c + 1)])
        else:
            nc.sync.dma_start(xt[:], x[128 * i:128 * (i + 1), :])
        Y = ystage.tile([128, DIM], f32, tag="Y")

        for g in range(8):           # groups of 4 feature blocks
            pt = psA.tile([128, 512], f32, tag="ptA")
            for j in range(4):
                b = 4 * g + j
                nc.tensor.transpose(
                    pt[:, 128 * j:128 * (j + 1)],
                    xt[:, 128 * b:128 * (b + 1)], idt[:])
            xT4 = sbst.tile([128, 512], f32, tag="xT4")
            copy(xT4[:], pt[:])
            pm = psB.tile([128, 512], f32, tag="pmA")
            for j in range(4):
                b = 4 * g + j
                nc.tensor.matmul(
                    pm[:, 128 * j:128 * (j + 1)],
                    xT4[:, 128 * j:128 * (j + 1)],
                    am[:, 128 * b:128 * (b + 1)],
                    start=True, stop=True)
            # scatter into Y: dest f~ = t*128 + pl*32 + (4g+j), src = j*128 + 4t + pl
            src = pm[:].rearrange("r (j t pl) -> r j t pl", j=4, t=32, pl=4)
            dst = Y[:].rearrange(
                "r (t pl g j) -> r g j t pl", t=32, pl=4, g=8, j=4)[:, g]
            copy(dst, src, kind="scatter")

        if i == 0:
            for c in range(8):
                nc.sync.dma_start(bm[:, 512 * c:512 * (c + 1)],
                                  bmats[:, 512 * c:512 * (c + 1)])
        O = ostage.tile([128, DIM], f32, tag="O")
        for g in range(8):           # groups of 4 f~ tiles
            pt = psA.tile([128, 512], f32, tag="ptA")
            for j in range(4):
                t = 4 * g + j
                nc.tensor.transpose(
                    pt[:, 128 * j:128 * (j + 1)],
                    Y[:, 128 * t:128 * (t + 1)], idt[:])
            z4 = sbst.tile([128, 512], f32, tag="xT4")
            copy(z4[:], pt[:])
            pm = psB.tile([128, 512], f32, tag="pmA")
            for j in range(4):
                t = 4 * g + j
                nc.tensor.matmul(
                    pm[:, 128 * j:128 * (j + 1)],
                    z4[:, 128 * j:128 * (j + 1)],
                    bm[:, 128 * t:128 * (t + 1)],
                    start=True, stop=True)
            # scatter to natural order: dest f = b*128 + 4t + pl = b*128 + 16g + 4j + pl
            src = pm[:].rearrange("r (j pl b) -> r j pl b", j=4, pl=4, b=32)
            dst = O[:].rearrange(
                "r (b g j pl) -> r g j pl b", b=32, g=8, j=4, pl=4)[:, g]
            copy(dst, src, kind="scatter")

        nc.sync.dma_start(out[128 * i:128 * (i + 1), :], O[:])


def _emit_kernel_v2(ctx, tc, out, x, amats, bmats, ident):
    """f32r weights-stationary variant: super-tiles of 256 rows, stage
    matmuls lhsT=matrix rhs=data at N=256 (f32r streams 1 cyc/row vs 4 for
    fp32), data kept feature-major between stages, f32r transposes (1.5
    cyc/row) for all shuffles after the first exact fp32 transpose."""
    import concourse.mybir as mybir

    nc = tc.nc
    f32 = mybir.dt.float32
    f32r = mybir.dt.float32r

    consts = ctx.enter_context(tc.tile_pool(name="consts", bufs=1))
    mstage = ctx.enter_context(tc.tile_pool(name="mstage", bufs=1))
    xin = ctx.enter_context(tc.tile_pool(name="xin", bufs=2))
    xTrp = ctx.enter_context(tc.tile_pool(name="xTrp", bufs=1))
    ypool = ctx.enter_context(tc.tile_pool(name="ypool", bufs=4))
    zpool = ctx.enter_context(tc.tile_pool(name="zpool", bufs=4))
    wpool = ctx.enter_context(tc.tile_pool(name="wpool", bufs=4))
    Ypool = ctx.enter_context(tc.tile_pool(name="Ypool", bufs=2))
    Opool = ctx.enter_context(tc.tile_pool(name="Opool", bufs=2))
    psT = ctx.enter_context(tc.tile_pool(name="psT", bufs=3, space="PSUM"))
    psM = ctx.enter_context(tc.tile_pool(name="psM", bufs=3, space="PSUM"))

    # constants: round matrices + identity to f32r on device
    amr = consts.tile([128, DIM], f32r, tag="amr")
    bmr = consts.tile([128, DIM], f32r, tag="bmr")
    idt = consts.tile([128, 128], f32, tag="idt")
    idtr = consts.tile([128, 128], f32r, tag="idtr")
    nc.sync.dma_start(idt[:], ident[:])
    nc.vector.tensor_copy(idtr[:], idt[:])
    am_st = mstage.tile([128, DIM], f32, tag="mst")
    for c in range(4):
        nc.sync.dma_start(am_st[:, 1024 * c:1024 * (c + 1)],
                          amats[:, 1024 * c:1024 * (c + 1)])
    for c in range(4):
        eng = nc.vector.tensor_copy if c % 2 else nc.scalar.copy
        eng(amr[:, 1024 * c:1024 * (c + 1)],
            am_st[:, 1024 * c:1024 * (c + 1)])
    bm_st = mstage.tile([128, DIM], f32, tag="mst")
    for c in range(4):
        nc.sync.dma_start(bm_st[:, 1024 * c:1024 * (c + 1)],
                          bmats[:, 1024 * c:1024 * (c + 1)])
    for c in range(4):
        eng = nc.vector.tensor_copy if c % 2 else nc.scalar.copy
        eng(bmr[:, 1024 * c:1024 * (c + 1)],
            bm_st[:, 1024 * c:1024 * (c + 1)])

    load = {"dve": 0.0, "act": 0.0}
    cost = {("dve", "plain"): 685, ("dve", "scatter"): 700,
            ("act", "plain"): 570, ("act", "scatter"): 1127}

    def copy(dst, src, kind="plain"):
        eng = min(("dve", "act"), key=lambda e: load[e] + cost[(e, kind)])
        load[eng] += cost[(eng, kind)]
        (nc.vector.tensor_copy if eng == "dve" else nc.scalar.copy)(dst, src)

    NST = NT // 2            # super-tiles of 256 rows
    for s in range(NST):
        # ---- T1: exact fp32 transposes x -> xTrBig [f', (b, c r-chunk)] f32r
        xTr = xTrp.tile([128, 32 * 256], f32r, tag="xTr")
        for c in range(2):
            xt = xin.tile([128, DIM], f32, tag="xt")
            nc.sync.dma_start(
                xt[:], x[256 * s + 128 * c:256 * s + 128 * (c + 1), :])
            for g in range(8):
                pt = psT.tile([128, 512], f32, tag="psT")
                for j in range(4):
                    b = 4 * g + j
                    nc.tensor.transpose(
                        pt[:, 128 * j:128 * (j + 1)],
                        xt[:, 128 * b:128 * (b + 1)], idt[:])
                # dest: col 256*(4g+j) + 128c + q
                dst = xTr[:].rearrange(
                    "f (bb cc q) -> f cc bb q", bb=32, cc=2, q=128)
                dst = dst[:, c, 4 * g:4 * g + 4]        # [128, 4, 128]
                src = pt[:].rearrange("f (j q) -> f j q", j=4, q=128)
                copy(dst, src)
        # ---- M1 + T2 interleaved per 4-block group: stage A f32r N=256,
        # then f32r transposes y -> Y_c rows-major (b-major contiguous)
        Ys = [Ypool.tile([128, DIM], f32r, tag="Y", name=f"Yc{c}")
              for c in range(2)]
        for g in range(8):
            ySBs = []
            for jj in range(2):
                q = 2 * g + jj
                pm = psM.tile([128, 512], f32, tag="psM")
                for j in range(2):
                    b = 2 * q + j
                    nc.tensor.matmul(
                        pm[:, 256 * j:256 * (j + 1)],
                        amr[:, 128 * b:128 * (b + 1)],
                        xTr[:, 256 * b:256 * (b + 1)],
                        start=True, stop=True)
                ySB = ypool.tile([128, 512], f32r, tag="ySB")
                copy(ySB[:], pm[:])
                ySBs.append(ySB)
            for c in range(2):
                pt = psT.tile([128, 512], f32r, tag="psT")
                for j in range(4):
                    b = 4 * g + j
                    jj, bb = b // 2 - 2 * g, b % 2
                    nc.tensor.transpose(
                        pt[:, 128 * j:128 * (j + 1)],
                        ySBs[jj][:, 256 * bb + 128 * c:256 * bb + 128 * (c + 1)],
                        idtr[:])
                # scatter into f~ order: dest = (p//4)*128 + (p%4)*32 + (4g+j)
                srcv = pt[:].rearrange(
                    "r (j tt pl) -> r j tt pl", j=4, tt=32, pl=4)
                dstv = Ys[c][:].rearrange(
                    "r (tt pl gg j) -> r gg j tt pl",
                    tt=32, pl=4, gg=8, j=4)[:, g]
                copy(dstv, srcv, kind="scatter")
        # ---- T3 + M2 + T4 interleaved per 4-tile group
        Os = [Opool.tile([128, DIM], f32, tag="O", name=f"Oc{c}")
              for c in range(2)]
        for g in range(8):
            wSBs = []
            for jj in range(2):
                q = 2 * g + jj
                pt = psT.tile([128, 512], f32r, tag="psT")
                for j in range(2):
                    t = 2 * q + j
                    for c in range(2):
                        nc.tensor.transpose(
                            pt[:, 256 * j + 128 * c:256 * j + 128 * (c + 1)],
                            Ys[c][:, 128 * t:128 * (t + 1)], idtr[:])
                zr = zpool.tile([128, 512], f32r, tag="zr")
                copy(zr[:], pt[:])
                pw = psM.tile([128, 512], f32, tag="psM")
                for j in range(2):
                    t = 2 * q + j
                    nc.tensor.matmul(
                        pw[:, 256 * j:256 * (j + 1)],
                        bmr[:, 128 * t:128 * (t + 1)],
                        zr[:, 256 * j:256 * (j + 1)],
                        start=True, stop=True)
                wSB = wpool.tile([128, 512], f32r, tag="wSB")
                copy(wSB[:], pw[:])
                wSBs.append(wSB)
            for c in range(2):
                pt = psT.tile([128, 512], f32r, tag="psT")
                for j in range(4):
                    t = 4 * g + j
                    jj, tt = t // 2 - 2 * g, t % 2
                    nc.tensor.transpose(
                        pt[:, 128 * j:128 * (j + 1)],
                        wSBs[jj][:, 256 * tt + 128 * c:256 * tt + 128 * (c + 1)],
                        idtr[:])
                # dest f = b*128 + 16g + 4j + pl ; src col = j*128 + pl*32 + b
                src = pt[:].rearrange("r (j pl b) -> r b j pl", j=4, pl=4, b=32)
                dst = Os[c][:].rearrange(
                    "r (b gg j pl) -> r gg b j pl", b=32, gg=8, j=4, pl=4)[:, g]
                copy(dst, src, kind="scatter")
        for c in range(2):
            nc.sync.dma_start(
                out[256 * s + 128 * c:256 * s + 128 * (c + 1), :], Os[c][:])


def _emit_kernel_v3(ctx, tc, out, x, cmats):
    """bf16 pipeline: DMA-XBAR transpose-load of x (bf16) straight into
    feature-major tiles, data-stationary bf16 matmuls for both stages, PE
    bf16 transposes for the mid f~ pivot writing bf16 PSUM (2x DVE copies),
    output DMA'd in f~ column order as bf16; host un-permutes + converts.

    f~ layout: f~ = 128*t + 4*b + pl  (p = 4t + pl). With this ordering the
    stage-A PSUM->SBUF scatter has 32-elem contiguous dest runs, which both
    DVE and ACT execute at plain-copy rates (4-elem runs are ~2x slower).
    """
    import concourse.mybir as mybir

    nc = tc.nc
    f32 = mybir.dt.float32
    bf16 = mybir.dt.bfloat16

    consts = ctx.enter_context(tc.tile_pool(name="consts", bufs=1))
    xin = ctx.enter_context(tc.tile_pool(name="xin", bufs=4))
    ypool = ctx.enter_context(tc.tile_pool(name="ypool", bufs=2))
    zpool = ctx.enter_context(tc.tile_pool(name="zpool", bufs=2))
    opool = ctx.enter_context(tc.tile_pool(name="opool", bufs=6))
    # PSUM budget (8 banks): psA 2x[128,1024]f32 = 4, psT 2x[128,1024]bf16
    # = 2, psB 2x[128,512]f32 = 2.
    psA = ctx.enter_context(tc.tile_pool(name="psA", bufs=2, space="PSUM"))
    psT = ctx.enter_context(tc.tile_pool(name="psT", bufs=2, space="PSUM"))
    psB = ctx.enter_context(tc.tile_pool(name="psB", bufs=2, space="PSUM"))

    # combined const dram tensor [idt(128) | am(4096) | bm(4096)]; idt+am
    # loaded up front in halves (needed by first stage-A), bm mid-stream.
    cm = consts.tile([128, 128 + 2 * DIM], bf16, tag="cmats")
    idt = cm[:, 0:128]
    am = cm[:, 128:128 + DIM]
    bm = cm[:, 128 + DIM:128 + 2 * DIM]
    nc.sync.dma_start(cm[:, 0:128 + DIM], cmats[:, 0:128 + DIM])

    # measured per-chunk costs (ns) on HW, greedy least-loaded assignment
    load = {"dve": 0.0, "act": 0.0}
    cost = {("dve", "c1_1024"): 1223, ("act", "c1_1024"): 1114,
            ("dve", "z1024"): 679, ("act", "z1024"): 996,
            ("dve", "c2_512"): 690, ("act", "c2_512"): 687}

    def copy(dst, src, kind):
        eng = min(("dve", "act"), key=lambda e: load[e] + cost[(e, kind)])
        load[eng] += cost[(eng, kind)]
        (nc.vector.tensor_copy if eng == "dve" else nc.scalar.copy)(dst, src)

    def load_x(i):
        # transpose-load: xT[p, b, r] = x[r, 128b + p], bf16
        xT = xin.tile([128, NB, 128], bf16, tag="xT", name=f"xT{i}")
        if i == 0:
            # split first tile's load so stage A can start ~4x earlier
            for q in range(4):
                nc.sync.dma_start_transpose(
                    xT[:, 8 * q:8 * (q + 1)],
                    x[0:128, 1024 * q:1024 * (q + 1)])
        elif i == 1:
            for q in range(2):
                nc.sync.dma_start_transpose(
                    xT[:, 16 * q:16 * (q + 1)],
                    x[128:256, 2048 * q:2048 * (q + 1)])
        else:
            nc.sync.dma_start_transpose(xT[:], x[128 * i:128 * (i + 1), :])
        return xT

    def stage_a(i, xT):
        # y[r, p] = x_b @ A_b; scatter to Y[f~ order] bf16
        Y = ypool.tile([128, DIM], bf16, tag="Y", name=f"Y{i}")
        for gp in range(4):
            pm = psA.tile([128, 1024], f32, tag="pmA")
            for j8 in range(8):
                b = 8 * gp + j8
                nc.tensor.matmul(
                    pm[:, 128 * j8:128 * (j8 + 1)],
                    xT[:, b], am[:, 128 * b:128 * (b + 1)],
                    start=True, stop=True)
            # dst col = 128t + 32gp + 4j8 + pl ; src col = 128j8 + 4t + pl
            src = pm[:].rearrange("r (jj t pl) -> r t jj pl",
                                  jj=8, t=32, pl=4)
            dst = Y[:].rearrange("r (t g jj pl) -> r g t jj pl",
                                 t=32, g=4, jj=8, pl=4)[:, gp]
            copy(dst, src, "c1_1024")
        return Y

    def stage_bt(i, Y):
        # mid pivot: z_t[4b+pl, r] = Y-tile-t^T (bf16 psum), then
        # stage B: out[r, f~ order] = z_t @ B_t, plain copy out.
        # Interleaved h-wise so M2 chunks start right after their z lands.
        Z = zpool.tile([128, DIM], bf16, tag="Z", name=f"Z{i}")
        O = opool.tile([128, DIM], bf16, tag="O", name=f"O{i}")
        for h in range(4):
            pt = psT.tile([128, 1024], bf16, tag="ptT")
            for j in range(8):
                t = 8 * h + j
                nc.tensor.transpose(
                    pt[:, 128 * j:128 * (j + 1)],
                    Y[:, 128 * t:128 * (t + 1)], idt[:])
            copy(Z[:, 1024 * h:1024 * (h + 1)], pt[:], "z1024")
        for g in range(8):
            pm = psB.tile([128, 512], f32, tag="pmB")
            for j in range(4):
                t = 4 * g + j
                nc.tensor.matmul(
                    pm[:, 128 * j:128 * (j + 1)],
                    Z[:, 128 * t:128 * (t + 1)],
                    bm[:, 128 * t:128 * (t + 1)],
                    start=True, stop=True)
            copy(O[:, 512 * g:512 * (g + 1)], pm[:], "c2_512")
        return O

    # out-DMAs for the first tiles are deferred until after every
    # transpose-load is issued: each DMA's transfer serializes against the
    # next DMA's start (scheduler models DMA engines as one device), so an
    # out between two loads delays the load stream and starves the PE.
    def emit_out(i, O, halves=1):
        if halves == 2:
            nc.sync.dma_start(out[128 * i:128 * (i + 1), 0:2048],
                              O[:, 0:2048])
            nc.sync.dma_start(out[128 * i:128 * (i + 1), 2048:4096],
                              O[:, 2048:4096])
        else:
            nc.sync.dma_start(out[128 * i:128 * (i + 1), :], O[:])

    deferred = []
    xT0 = load_x(0)
    Y0 = stage_a(0, xT0)
    nc.sync.dma_start(cm[:, 128 + DIM:], cmats[:, 128 + DIM:])
    deferred.append((0, stage_bt(0, Y0)))
    for i in range(1, NT):
        xT = load_x(i)
        if i == NT - 1:
            for k, O in deferred:
                emit_out(k, O)
            deferred = []
        Y = stage_a(i, xT)
        O = stage_bt(i, Y)
        if i < 4:
            deferred.append((i, O))
        else:
            emit_out(i, O, halves=2 if i == NT - 1 else 1)


def _hoist_matmul_waits(nc):
    """Walrus's fp32/transpose matmul (self-loading LDWEIGHTS) accepts fewer
    sync waits than Tile may assign. Hoist multi-waits onto a PE NoOp inserted
    just before the matmul — same engine queue, so ordering is identical."""
    import concourse.mybir as mybir

    n_hoisted = 0
    for blk in nc.m.functions[0].blocks:
        il = blk.instructions
        i = 0
        while i < len(il):
            inst = il[i]
            si = inst.sync_info
            if (si is not None and len(si.on_wait) > 1
                    and not isinstance(inst, mybir.InstNoOp)):
                waits = list(si.on_wait)
                # keep the last wait on the matmul; one NoOp per extra wait
                # (cayman instructions carry at most one sem-wait each)
                for k, w in enumerate(waits[:-1]):
                    nop = mybir.InstNoOp(
                        name=f"{inst.name}_hw{k}", engine=inst.engine,
                        bass_nofuse=True)
                    nop.sync_info = mybir.SyncInfo(on_wait=[w], on_update=[])
                    nc.register_instruction(nop, overwrite=True)
                    il.insert(i, nop)
                    i += 1
                    n_hoisted += 1
                inst.sync_info = mybir.SyncInfo(
                    on_wait=[waits[-1]], on_update=list(si.on_update))
            i += 1
    return n_hoisted


_CACHED = {}
VARIANT = "v3"   # "v1" fused-fp32 | "v2" f32r | "v3" bf16 dma-transpose


def _build_bass(variant=None):
    variant = variant or VARIANT
    if variant in _CACHED:
        return _CACHED[variant]
    from contextlib import ExitStack
    import concourse.bass as bass
    import concourse.tile as tile
    import concourse.mybir as mybir

    f32 = mybir.dt.float32
    bf16 = mybir.dt.bfloat16
    dt = bf16 if variant == "v3" else f32
    nc = bass.Bass("TRN2", target_bir_lowering=False, debug=False,
                   num_devices=NCORES)
    x = nc.dram_tensor("x", [RPC, DIM], dt, kind="ExternalInput").ap()
    out = nc.dram_tensor("out", [RPC, DIM], dt, kind="ExternalOutput").ap()

    with tile.TileContext(nc) as tc:
        with ExitStack() as ctx:
            if variant == "v3":
                cmats = nc.dram_tensor("cmats", [128, 128 + 2 * DIM], dt,
                                       kind="ExternalInput").ap()
                _emit_kernel_v3(ctx, tc, out, x, cmats)
            else:
                amats = nc.dram_tensor(
                    "amats", [128, DIM], dt, kind="ExternalInput").ap()
                bmats = nc.dram_tensor(
                    "bmats", [128, DIM], dt, kind="ExternalInput").ap()
                ident = nc.dram_tensor(
                    "ident", [128, 128], dt, kind="ExternalInput").ap()
                emit = {"v1": _emit_kernel, "v2": _emit_kernel_v2}[variant]
                emit(ctx, tc, out, x, amats, bmats, ident)

    _hoist_matmul_waits(nc)
    _CACHED[variant] = nc
    return nc


def _build_mats2(angles):
    """v3 matrices. amats as in _build_mats; bmats with the f~ internal
    layout q = 4b + pl: bmats[4b+pl, 128t + 4b' + pl] = B_{4t+pl}[b, b']."""
    angles = np.asarray(angles, np.float64)
    amats = np.zeros((128, DIM), np.float64)
    for b in range(NB):
        amats[:, 128 * b:128 * b + 128] = _butterfly_np(
            np.eye(128), angles[0:7, 64 * b:64 * b + 64])
    bmats = np.zeros((128, DIM), np.float64)
    for t in range(32):
        for pl in range(4):
            p = 4 * t + pl
            BpT = _butterfly_np(np.eye(32), angles[7:12, p::128])
            for bb in range(32):
                bmats[4 * bb + pl,
                      128 * t + 4 * np.arange(32) + pl] = BpT[bb]
    return amats.astype(np.float32), bmats.astype(np.float32)


def _ftilde_perm():
    """natural-order column f -> column index in the v3 device output.
    natural f = 128 b' + p, p = 4t + pl; device col = 128 t + 4 b' + pl."""
    f = np.arange(DIM)
    b, rem = f // 128, f % 128
    t, pl = rem // 4, rem % 4
    return 128 * t + 4 * b + pl


def make_in_maps(x, angles, variant=None):
    variant = variant or VARIANT
    if variant == "v3":
        import ml_dtypes
        amats, bmats = _build_mats2(angles)
        npdt = ml_dtypes.bfloat16
        x = np.ascontiguousarray(np.asarray(x, np.float32).astype(npdt))
        cmats = np.concatenate(
            [np.eye(128, dtype=np.float32), amats, bmats],
            axis=1).astype(npdt)
        return [{"x": x[c * RPC:(c + 1) * RPC], "cmats": cmats}
                for c in range(NCORES)]
    amats, bmats = _build_mats(angles)
    npdt = np.float32
    x = np.ascontiguousarray(np.asarray(x, np.float32).astype(npdt))
    amats = amats.astype(npdt)
    bmats = bmats.astype(npdt)
    ident = np.eye(128, dtype=npdt)
    return [
        {"x": x[c * RPC:(c + 1) * RPC], "amats": amats, "bmats": bmats,
         "ident": ident}
        for c in range(NCORES)
    ]


def run_on_hw(x, angles, trace=False, trace_kwargs=None):
    from concourse.bass_utils import run_bass_kernel_spmd
    nc = _build_bass()
    in_maps = make_in_maps(x, angles)
    res = run_bass_kernel_spmd(
        nc, in_maps, core_ids=list(range(NCORES)), trace=trace,
        **(trace_kwargs or {}))
    out = np.concatenate([res.results[c]["out"] for c in range(NCORES)], axis=0)
    if VARIANT == "v3":
        out = out[:, _ftilde_perm()].astype(np.float32)
    return out, res


def kernel(x, angles):
    last_err = None
    for attempt in range(3):
        try:
            out, _ = run_on_hw(x, angles, trace=False)
            return np.ascontiguousarray(out.astype(np.float32))
        except Exception as e:  # transient NRT/device errors: retry
            last_err = e
            import time
            time.sleep(5)
    raise last_err

